# revision 26
# baseline (speedup 1.0000x reference)
"""Trainium2 Bass kernel for nn_MultiHeadedAttention (B=2, S=2048, D=1024, H=16).

Sharding: batch (2) x head-groups (4) -> 8 cores. Core c handles batch c//4,
heads [4*(c%4), 4*(c%4)+4). Per core everything runs in transposed layouts:

  phase 1: Q^T = Wq_s x_q^T, K^T = Wk_s x_k^T (features on partitions), and
           V in natural [seq, feat] layout with a ones-column appended per head.
  phase 2: per (head, q-block): S^T = K^T(tile)^T-contract Q^T  (k on
           partitions), exp on ScalarE with the 1/sqrt(dk) scale folded in
           (softmax without max-subtraction: |scores| <~ 8, safe range),
           then PV via matmul with the ones-column producing the softmax
           denominators as row 64 of the accumulator.  Normalization uses
           reciprocal + gpsimd partition_broadcast + one vector multiply.
  phase 3: y^T partial = Wo_s^T x_attn^T; host sums the 4 partials per batch.

Matmul operands are bf16 (fp32 PSUM accumulation); fp32 everywhere else.
The tiny t-bias MLP ([B,1,1,1] -> [B,64]) is folded into the K projection
bias on the host during input sharding.
"""

import numpy as np

B, S, D, H, DK = 2, 2048, 1024, 16, 64
HPC = 4            # heads per core
DPC = HPC * DK     # 256 features per core
NCORES = 8

TRACE = False          # test harness sets True to capture an NTFF profile
LAST_EXEC_NS = None    # filled when TRACE
LAST_RESULTS = None

_BUILT = None


def _install_ntff_shim():
    """antenv.axon_hooks is absent in this image; recreate it so trace=True
    can ship NTFF profiles back through the axon tunnel."""
    import sys, types
    try:
        from antenv import axon_hooks  # noqa: F401
        return
    except ImportError:
        pass
    import antenv
    mod = types.ModuleType("antenv.axon_hooks")
    _hook = [None]
    mod.set_axon_ntff_profile_hook = lambda h: _hook.__setitem__(0, h)
    mod.get_axon_ntff_profile_hook = lambda: _hook[0]
    sys.modules["antenv.axon_hooks"] = mod
    antenv.axon_hooks = mod
    try:
        from trn_agent_boot.trn_boot import _ntff_profile_via_ctypes
        mod.set_axon_ntff_profile_hook(
            _ntff_profile_via_ctypes("/opt/axon/libaxon_pjrt.so"))
    except Exception:
        pass


def _build():
    """Build the per-core Bass graph (identical on all 8 cores)."""
    import concourse.tile as tile
    from concourse import mybir, bacc

    f32 = mybir.dt.float32
    bf16 = mybir.dt.bfloat16

    nc = bacc.Bacc()

    xq_t = nc.dram_tensor("xq_t", [D, S], bf16, kind="ExternalInput")
    xk_t = nc.dram_tensor("xk_t", [D, S], bf16, kind="ExternalInput")
    xv_t = nc.dram_tensor("xv_t", [D, S], bf16, kind="ExternalInput")
    wq_t = nc.dram_tensor("wq_t", [D, DPC], bf16, kind="ExternalInput")
    wk_t = nc.dram_tensor("wk_t", [D, DPC], bf16, kind="ExternalInput")
    wv_t = nc.dram_tensor("wv_t", [D, DPC], bf16, kind="ExternalInput")
    wo_t = nc.dram_tensor("wo_t", [DPC, D], bf16, kind="ExternalInput")
    bq2 = nc.dram_tensor("bq2", [HPC, DK], f32, kind="ExternalInput")
    bk2 = nc.dram_tensor("bk2", [HPC, DK], f32, kind="ExternalInput")
    bv1 = nc.dram_tensor("bv1", [1, DPC], f32, kind="ExternalInput")
    bo8 = nc.dram_tensor("bo8", [8, 128], f32, kind="ExternalInput")
    y_t = nc.dram_tensor("y_t", [D, S], f32, kind="ExternalOutput")

    NB = 2          # phase-1 seq blocks of 1024
    BW = S // NB
    JBLOCKS = [(0, 1024), (1024, 1024)]
    NJ = len(JBLOCKS)
    NE = D // 128   # 8 feature chunks
    NST = S // 128  # 16 seq tiles of 128 (k tiles)

    with tile.TileContext(nc) as tc:
        with tc.tile_pool(name="consts", bufs=1) as consts, \
             tc.tile_pool(name="persist", bufs=1) as persist:

            # ---- constants ----
            wq_sb = consts.tile([128, NE, DPC], bf16, tag="wq")
            wk_sb = consts.tile([128, NE, DPC], bf16, tag="wk")
            wv_sb = consts.tile([128, NE, DPC], bf16, tag="wv")
            nc.sync.dma_start(wq_sb[:, :, :],
                              wq_t.rearrange("(e p) n -> p e n", p=128))
            nc.sync.dma_start(wk_sb[:, :, :],
                              wk_t.rearrange("(e p) n -> p e n", p=128))
            nc.sync.dma_start(wv_sb[:, :, :],
                              wv_t.rearrange("(e p) n -> p e n", p=128))
            wo_sb = consts.tile([128, 2, D], bf16, tag="wo")
            nc.sync.dma_start(wo_sb[:, :, :],
                              wo_t.rearrange("(f p) n -> p f n", p=128))
            bq_sb = consts.tile([64, HPC], f32, tag="bq")
            bk_sb = consts.tile([64, HPC], f32, tag="bk")
            nc.sync.dma_start(bq_sb[:, :], bq2.rearrange("h p -> p h"))
            nc.sync.dma_start(bk_sb[:, :], bk2.rearrange("h p -> p h"))
            bo_sb = consts.tile([128, 8], f32, tag="bo")
            nc.sync.dma_start(bo_sb[:, :], bo8.rearrange("o p -> p o"))
            bv_row = consts.tile([1, DPC], f32, tag="bvr")
            nc.sync.dma_start(bv_row[0:1, :], bv1[0:1, :])
            bv_bc = consts.tile([128, HPC, DK], f32, tag="bvb")
            nc.gpsimd.partition_broadcast(
                bv_bc.rearrange("p h d -> p (h d)"), bv_row[0:1, :])

            # ---- persistent activations ----
            # per-head tiles, head dims at rows 0..63; rows 64..127 zeroed so
            # scores matmuls can use K=128 (FWL-eligible weights)
            qt_sb = persist.tile([128, HPC, S], bf16, tag="qt")
            kt_sb = persist.tile([128, HPC, S], bf16, tag="kt")
            nc.vector.memset(qt_sb[64:128, :, :], 0.0)
            nc.vector.memset(kt_sb[64:128, :, :], 0.0)
            v_sb = persist.tile([128, NST, HPC, DK + 1], bf16, tag="v")
            # early-attention P staging: heads 0-1, k-tiles 0-7 of q-block 0
            pe_sb = persist.tile([128, 2, 8, 1024], bf16, tag="pearly")
            ones1 = consts.tile([128, 1], f32, tag="ones1")
            nc.vector.memset(ones1[:, :], 1.0)
            nc.vector.tensor_copy(
                v_sb[:, :, :, DK:DK + 1].rearrange("p a b c -> p (a b c)"),
                ones1[:, 0:1].broadcast_to([128, NST * HPC]))

            # ================= phase 1: projections =================
            with tc.tile_pool(name="xin", bufs=2) as xin, \
                 tc.tile_pool(name="proj_ps", bufs=2, space="PSUM") as proj_ps, \
                 tc.tile_pool(name="v_ps", bufs=2, space="PSUM") as v_ps:
                for q in range(NB):
                    qs = slice(q * BW, (q + 1) * BW)
                    xq_q = xin.tile([128, NE, BW], bf16, tag="xq")
                    xk_q = xin.tile([128, NE, BW], bf16, tag="xk")
                    xv_q = xin.tile([128, NE, BW], bf16, tag="xv")
                    for eh in range(2):
                        eslc = slice(eh * 4, eh * 4 + 4)
                        nc.sync.dma_start(
                            xq_q[:, eslc, :],
                            xq_t.rearrange("(e p) s -> p e s",
                                           p=128)[:, eslc, qs])
                    for eh in range(2):
                        eslc = slice(eh * 4, eh * 4 + 4)
                        nc.sync.dma_start(
                            xk_q[:, eslc, :],
                            xk_t.rearrange("(e p) s -> p e s",
                                           p=128)[:, eslc, qs])
                    for eh in range(2):
                        eslc = slice(eh * 4, eh * 4 + 4)
                        nc.sync.dma_start(
                            xv_q[:, eslc, :],
                            xv_t.rearrange("(e p) s -> p e s",
                                           p=128)[:, eslc, qs])
                    for m in range(2):
                        ms = slice(m * 128, (m + 1) * 128)
                        ps = proj_ps.tile([128, BW], f32, tag="proj")
                        for e in range(NE):
                            for hf in range(2):
                                hs = slice(hf * 512, hf * 512 + 512)
                                nc.tensor.matmul(ps[:, hs], wq_sb[:, e, ms],
                                                 xq_q[:, e, hs],
                                                 start=(e == 0),
                                                 stop=(e == NE - 1))
                        nc.vector.tensor_scalar_add(
                            qt_sb[0:64, 2 * m, qs], ps[0:64, :],
                            bq_sb[:, 2 * m:2 * m + 1])
                        nc.vector.tensor_scalar_add(
                            qt_sb[0:64, 2 * m + 1, qs], ps[64:128, :],
                            bq_sb[:, 2 * m + 1:2 * m + 2])
                        ps = proj_ps.tile([128, BW], f32, tag="proj")
                        for e in range(NE):
                            for hf in range(2):
                                hs = slice(hf * 512, hf * 512 + 512)
                                nc.tensor.matmul(ps[:, hs], wk_sb[:, e, ms],
                                                 xk_q[:, e, hs],
                                                 start=(e == 0),
                                                 stop=(e == NE - 1))
                        nc.vector.tensor_scalar_add(
                            kt_sb[0:64, 2 * m, qs], ps[0:64, :],
                            bk_sb[:, 2 * m:2 * m + 1])
                        nc.vector.tensor_scalar_add(
                            kt_sb[0:64, 2 * m + 1, qs], ps[64:128, :],
                            bk_sb[:, 2 * m + 1:2 * m + 2])
                    if q == 0:
                        # early scores+exp for heads 0-1 over block-0 k-tiles;
                        # PV is deferred to phase 2 (PSUM stays free for proj)
                        with tc.tile_pool(name="sc_e", bufs=1,
                                          space="PSUM") as sc_e:
                            for eh in range(2):
                                for i in range(8):
                                    ks = slice(i * 128, (i + 1) * 128)
                                    s_ps = sc_e.tile([128, 1024], f32, tag="se")
                                    for half in range(2):
                                        hs = slice(half * 512, half * 512 + 512)
                                        nc.tensor.matmul(
                                            s_ps[:, hs],
                                            kt_sb[:, eh, ks],
                                            qt_sb[:, eh, hs],
                                            start=True, stop=True)
                                    nc.scalar.activation(
                                        pe_sb[:, eh, i, :], s_ps[:, :],
                                        mybir.ActivationFunctionType.Exp,
                                        scale=0.125)
                    for st in range(8):
                        stg = q * 8 + st
                        ps = v_ps.tile([128, DPC], f32, tag="vps")
                        for e in range(NE):
                            nc.tensor.matmul(
                                ps[:, :],
                                xv_q[:, e, st * 128:(st + 1) * 128],
                                wv_sb[:, e, :],
                                start=(e == 0), stop=(e == NE - 1))
                        nc.vector.tensor_tensor(
                            out=v_sb[:, stg, :, 0:DK],
                            in0=ps.rearrange("p (h d) -> p h d", h=HPC),
                            in1=bv_bc[:, :, :],
                            op=mybir.AluOpType.add)

            # ================= phase 2+3: attention + out proj =================
            KG = 4            # k-tiles per exp batch
            with tc.tile_pool(name="xattn", bufs=1) as xattn_pool, \
                 tc.tile_pool(name="psb", bufs=4) as p_pool, \
                 tc.tile_pool(name="oasb", bufs=4) as oa_pool, \
                 tc.tile_pool(name="rsb", bufs=2) as r_pool, \
                 tc.tile_pool(name="rbsb", bufs=2) as rb_pool, \
                 tc.tile_pool(name="ysb", bufs=3) as y_pool, \
                 tc.tile_pool(name="drs", bufs=2, space="DRAM") as dr_pool, \
                 tc.tile_pool(name="sc_ps", bufs=2, space="PSUM") as sc_ps, \
                 tc.tile_pool(name="oy_ps", bufs=2, space="PSUM") as oy_ps:

                xa_sb = xattn_pool.tile([128, 2, S], bf16, tag="xa")

                def emit_y(J, o_list):
                    yJ0, yJW = JBLOCKS[J]
                    yJs = slice(yJ0, yJ0 + yJW)
                    for o in o_list:
                        os_ = slice(o * 128, (o + 1) * 128)
                        y_sb = y_pool.tile([128, yJW], f32, tag="y")
                        ps = oy_ps.tile([128, yJW], f32, tag="oy")
                        for f in range(2):
                            for half in range(yJW // 512):
                                hs = slice(half * 512, half * 512 + 512)
                                jj = slice(yJ0 + half * 512,
                                           yJ0 + half * 512 + 512)
                                nc.tensor.matmul(ps[:, hs],
                                                 wo_sb[:, f, os_],
                                                 xa_sb[:, f, jj],
                                                 start=(f == 0), stop=(f == 1))
                        nc.vector.tensor_scalar_add(
                            y_sb[:, :], ps[:, :], bo_sb[:, o:o + 1])
                        nc.sync.dma_start(y_t[os_, yJs], y_sb[:, :])

                for J in range(NJ):
                    J0, JW = JBLOCKS[J]
                    Js = slice(J0, J0 + JW)
                    for h in ([2, 3, 0, 1] if J == 0 else range(HPC)):
                        pb = 64 * (h % 2)
                        hp = slice(pb, pb + DK)
                        m = h // 2
                        o_ps = oy_ps.tile([DK + 1, JW], f32, tag="oy")
                        early = J == 0 and h < 2
                        iorder = (list(range(8, NST)) + list(range(8))
                                  if early else list(range(NST)))
                        for ipos, i in enumerate(iorder):
                            ks = slice(i * 128, (i + 1) * 128)
                            if early and i < 8:
                                p_sb = pe_sb[:, h, i, :]
                            else:
                                s_ps = sc_ps.tile([128, JW], f32, tag="sc")
                                for half in range(JW // 512):
                                    hs = slice(half * 512, half * 512 + 512)
                                    jj = slice(J0 + half * 512,
                                               J0 + half * 512 + 512)
                                    nc.tensor.matmul(s_ps[:, hs],
                                                     kt_sb[:, h, ks],
                                                     qt_sb[:, h, jj],
                                                     start=True, stop=True)
                                p_sb = p_pool.tile([128, JW], bf16, tag="p")
                                nc.scalar.activation(
                                    p_sb[:, :], s_ps[:, :],
                                    mybir.ActivationFunctionType.Exp,
                                    scale=0.125)
                            for half in range(JW // 512):
                                hs = slice(half * 512, half * 512 + 512)
                                nc.tensor.matmul(o_ps[:, hs],
                                                 v_sb[:, i, h, :],
                                                 p_sb[:, hs],
                                                 start=(ipos == 0),
                                                 stop=(ipos == NST - 1))
                        # fast evacuation so the PSUM slot frees immediately
                        oa_sb = oa_pool.tile([DK + 1, JW], f32, tag="oa")
                        nc.vector.tensor_copy(oa_sb[:, :], o_ps[:, :])
                        # reciprocal on a [128, JW/128] reshape (lane-parallel)
                        # via a DRAM bounce (SBUF APs can't repartition)
                        d1 = dr_pool.tile([1, JW], f32, tag="d1")
                        nc.gpsimd.dma_start(d1[0:1, :], oa_sb[DK:DK + 1, :])
                        r2 = r_pool.tile([128, JW // 128], f32, tag="r2")
                        nc.gpsimd.dma_start(
                            r2[:, :],
                            d1.rearrange("o (p f) -> (o p) f", p=128))
                        nc.vector.reciprocal(r2[:, :], r2[:, :])
                        d2 = dr_pool.tile([128, JW // 128], f32, tag="d2")
                        nc.gpsimd.dma_start(d2[:, :], r2[:, :])
                        rb_sb = rb_pool.tile([64, JW], f32, tag="rb")
                        nc.gpsimd.dma_start(
                            rb_sb[:, :],
                            d2.rearrange("p f -> (p f)").unsqueeze(0)
                              .broadcast_to([64, JW]))
                        nc.vector.tensor_tensor(
                            out=xa_sb[hp, m, Js], in0=oa_sb[0:DK, :],
                            in1=rb_sb[:, :], op=mybir.AluOpType.mult)
                        if J > 0:
                            emit_y(J - 1, [2 * h, 2 * h + 1])
                    # y-projection of the previous q-block is interleaved
                    # into this block's attention by the per-head hook above
                emit_y(NJ - 1, list(range(8)))

    nc.finalize()
    return nc


def _get_built():
    global _BUILT
    if _BUILT is None:
        _BUILT = _build()
    return _BUILT


def kernel(**inputs):
    global LAST_EXEC_NS, LAST_RESULTS
    import ml_dtypes
    from concourse import bass_utils

    bf16 = ml_dtypes.bfloat16
    inp = {k: np.ascontiguousarray(np.asarray(v), dtype=np.float32)
           for k, v in inputs.items()}

    # host: t-bias MLP, folded into the K-projection bias
    t = inp["t"].reshape(B)
    h1 = np.maximum(inp["tW1"][:, 0][None, :] * t[:, None] + inp["tb1"][None, :], 0.0)
    t_bias = h1 @ inp["tW2"].T + inp["tb2"][None, :]          # [B, DK]

    in_maps = []
    for c in range(NCORES):
        b, g = c // 4, c % 4
        sl = slice(g * DPC, (g + 1) * DPC)
        bo_full = inp["bo"] if g == 0 else np.zeros(D, np.float32)
        in_maps.append({
            "xq_t": np.ascontiguousarray(inp["query"][b].T.astype(bf16)),
            "xk_t": np.ascontiguousarray(inp["key"][b].T.astype(bf16)),
            "xv_t": np.ascontiguousarray(inp["value"][b].T.astype(bf16)),
            "wq_t": np.ascontiguousarray(inp["Wq"][sl, :].T.astype(bf16)),
            "wk_t": np.ascontiguousarray(inp["Wk"][sl, :].T.astype(bf16)),
            "wv_t": np.ascontiguousarray(inp["Wv"][sl, :].T.astype(bf16)),
            "wo_t": np.ascontiguousarray(inp["Wo"][:, sl].T.astype(bf16)),
            "bq2": inp["bq"][sl].reshape(HPC, DK).copy(),
            "bk2": (inp["bk"][sl] + np.tile(t_bias[b], HPC)).reshape(HPC, DK),
            "bv1": inp["bv"][sl].reshape(1, DPC).copy(),
            "bo8": bo_full.reshape(8, 128).copy(),
        })

    nc = _get_built()
    if TRACE:
        _install_ntff_shim()
    try:
        res = bass_utils.run_bass_kernel_spmd(
            nc, in_maps, core_ids=list(range(NCORES)), trace=TRACE)
    except Exception:
        # transient device-unrecoverable states have been observed on a
        # first run; one retry on a fresh execute context clears them
        import time
        time.sleep(2.0)
        res = bass_utils.run_bass_kernel_spmd(
            nc, in_maps, core_ids=list(range(NCORES)), trace=False)
    LAST_EXEC_NS = res.exec_time_ns
    LAST_RESULTS = res

    out = np.zeros((B, S, D), np.float32)
    for c in range(NCORES):
        out[c // 4] += res.results[c]["y_t"].T
    return out
